# revision 8
# baseline (speedup 1.0000x reference)
"""Trainium2 Bass kernel for nn_Attention (B=4, S=1024, DIM=1024, H=16, Dh=64).

Sharding: 8 cores = 4 batches x 2 head-groups (8 heads / 512 inner channels
each).  Each core computes q/k/v projections for its head shard, RoPE,
attention, and a partial output projection (its rows of Wo); the host sums
the two head-group partials per batch (the tensor-parallel all-reduce done
on host) and concatenates batches.

Device dataflow (per core), all matmuls in float32r (TF32-like, 1 cyc/row):
  x^T staged in SBUF ->
  Q^T,K^T = W^T @ x^T  (+bias via K=1 matmul into the PSUM accumulation)
  RoPE in transposed layout: qr = q*cos + P_rot@(q*sin)  (P_rot on PE)
  scores^T[k,q] = K_h @ Q_h^T   (K=64; two heads concurrently via
                                 tile_position row groups 0/64)
  P^T = exp(scores^T/8 + maskbias[k])   (ACT; key mask folded into exp bias)
  attn^T[c,q] (+rowsum row via a ones-column in V_aug) = V_aug^T @ P^T
  normalize by 1/rowsum (DVE recip + PE partition-broadcast)
  out[q,:] = attn^T.T @ Wo_shard + bo/2, masked rows zeroed on the
  PSUM->SBUF copy.
"""

import numpy as np

B, S, DIM, HEADS, HEAD_DIM = 4, 1024, 1024, 16, 64
INNER = HEADS * HEAD_DIM
HG = 2                      # head groups (tensor-parallel shards)
DSH = INNER // HG           # 512 inner channels per core
HSH = HEADS // HG           # 8 heads per core
NCORES = B * HG
KT = DIM // 128             # 8 contraction tiles
MT = DSH // 128             # 4 output row tiles for Q^T/K^T
ST = S // 128               # 8 seq tiles
NC2 = S // 512              # 2 free-dim chunks of 512
MASK_NEG = -80.0

_CACHE = {}


def _build():
    import concourse.tile as tile
    from concourse import bacc, mybir

    f32 = mybir.dt.float32
    f32r = mybir.dt.float32r
    AF = mybir.ActivationFunctionType
    OP = mybir.AluOpType

    nc = bacc.Bacc("TRN2", target_bir_lowering=False, debug=False)

    xT_d = nc.dram_tensor("xT", [128, KT, S], f32r, kind="ExternalInput")
    wq_d = nc.dram_tensor("wq", [128, KT, MT, 128], f32r, kind="ExternalInput")
    wk_d = nc.dram_tensor("wk", [128, KT, MT, 128], f32r, kind="ExternalInput")
    wv_d = nc.dram_tensor("wv", [128, KT, DSH], f32r, kind="ExternalInput")
    wo_d = nc.dram_tensor("wo", [128, MT, DIM], f32r, kind="ExternalInput")
    bq_d = nc.dram_tensor("bq", [1, DSH], f32r, kind="ExternalInput")
    bk_d = nc.dram_tensor("bk", [1, DSH], f32r, kind="ExternalInput")
    bv_d = nc.dram_tensor("bv", [1, DSH], f32r, kind="ExternalInput")
    bo_d = nc.dram_tensor("bo", [1, DIM], f32r, kind="ExternalInput")
    cos_d = nc.dram_tensor("cos2", [128, S], f32, kind="ExternalInput")
    sin_d = nc.dram_tensor("sin2", [128, S], f32, kind="ExternalInput")
    prt_d = nc.dram_tensor("prt", [128, 128], f32r, kind="ExternalInput")
    maskb_d = nc.dram_tensor("maskb", [128, ST], f32, kind="ExternalInput")
    mask01_d = nc.dram_tensor("mask01", [128, ST], f32, kind="ExternalInput")
    out_d = nc.dram_tensor("out", [S, DIM], f32, kind="ExternalOutput")

    with tile.TileContext(nc) as tc, \
         tc.tile_pool(name="persist", bufs=1) as persist:
        with tc.tile_pool(name="w1", bufs=1) as w1:
            # phase-1-only constants
            xT = w1.tile([128, KT, S], f32r)
            wq = w1.tile([128, KT, MT, 128], f32r)
            wk = w1.tile([128, KT, MT, 128], f32r)
            wv = w1.tile([128, KT, DSH], f32r)
            bq = w1.tile([1, DSH], f32r)
            bk = w1.tile([1, DSH], f32r)
            bv = w1.tile([1, DSH], f32r)
            cos2 = w1.tile([128, S], f32)
            sin2 = w1.tile([128, S], f32)
            prt = w1.tile([128, 128], f32r)
            for t, d in [(xT, xT_d), (wq, wq_d), (wk, wk_d), (wv, wv_d),
                         (bq, bq_d), (bk, bk_d), (bv, bv_d),
                         (cos2, cos_d), (sin2, sin_d), (prt, prt_d)]:
                nc.sync.dma_start(out=t[:], in_=d.ap())
            # persistent across phases
            wo = persist.tile([128, MT, DIM], f32r)
            bo = persist.tile([1, DIM], f32r)
            maskb = persist.tile([128, ST], f32)
            mask01 = persist.tile([128, ST], f32)
            ones = persist.tile([1, S], f32r)
            for t, d in [(wo, wo_d), (bo, bo_d), (maskb, maskb_d),
                         (mask01, mask01_d)]:
                nc.sync.dma_start(out=t[:], in_=d.ap())
            ones_f = w1.tile([128, S], f32)
            nc.vector.memset(ones_f[:], 1.0)
            nc.vector.tensor_copy(ones[:], ones_f[0:1, :])

            qT = persist.tile([128, MT, S], f32r)
            kT = persist.tile([128, MT, S], f32r)
            vaug = persist.tile([128, ST, HSH, HEAD_DIM + 1], f32r)
            nc.vector.tensor_copy(
                vaug[:, :, :, HEAD_DIM:HEAD_DIM + 1],
                ones_f[:, 0:ST * HSH].rearrange(
                    "p (a b c) -> p a b c", b=HSH, c=1))

            # ---- phase 1: projections + RoPE -------------------------
            with tc.tile_pool(name="p1ps", bufs=4, space="PSUM") as p1ps, \
                 tc.tile_pool(name="p1pp", bufs=3, space="PSUM") as p1pp, \
                 tc.tile_pool(name="p1sb", bufs=3) as p1sb:
                # V first (PV needs all of it)
                for st in range(ST):
                    ps = p1ps.tile([128, DSH], f32, tag="ps")
                    nc.tensor.matmul(out=ps[:], lhsT=ones[0:1, 0:128],
                                     rhs=bv[:], start=True, stop=False)
                    for kt in range(KT):
                        nc.tensor.matmul(
                            out=ps[:],
                            lhsT=xT[:, kt, st * 128:(st + 1) * 128],
                            rhs=wv[:, kt, :],
                            start=False, stop=(kt == KT - 1))
                    nc.vector.tensor_copy(
                        vaug[:, st, :, 0:HEAD_DIM],
                        ps[:].rearrange("p (h d) -> p h d", h=HSH))

                # K then Q, per row-tile, in 512-wide column chunks
                for dst, w, b in ((kT, wk, bk), (qT, wq, bq)):
                    for mt in range(MT):
                        for c2 in range(NC2):
                            sl = slice(c2 * 512, (c2 + 1) * 512)
                            ps = p1ps.tile([128, 512], f32, tag="ps")
                            nc.tensor.matmul(
                                out=ps[:],
                                lhsT=b[0:1, mt * 128:(mt + 1) * 128],
                                rhs=ones[0:1, sl], start=True, stop=False)
                            for kt in range(KT):
                                nc.tensor.matmul(
                                    out=ps[:],
                                    lhsT=w[:, kt, mt, :],
                                    rhs=xT[:, kt, sl],
                                    start=False, stop=(kt == KT - 1))
                            if mt == 0:
                                # only the first 64 flat channels are RoPE'd
                                # (reference rotates rot_dim=64 of the flat
                                # 1024-dim vector); rows 64-127 get identity
                                # via cos=1/sin=0 from the host.
                                sinp = p1sb.tile([128, 512], f32r, tag="sinp")
                                nc.vector.tensor_tensor(
                                    sinp[:], ps[:], sin2[:, sl], op=OP.mult)
                                cosp = p1sb.tile([128, 512], f32, tag="cosp")
                                nc.vector.tensor_tensor(
                                    cosp[:], ps[:], cos2[:, sl], op=OP.mult)
                                pp = p1pp.tile([128, 512], f32, tag="pp")
                                nc.tensor.matmul(out=pp[:], lhsT=prt[:],
                                                 rhs=sinp[:],
                                                 start=True, stop=True)
                                nc.vector.tensor_tensor(
                                    dst[:, mt, sl], cosp[:], pp[:], op=OP.add)
                            else:
                                nc.vector.tensor_copy(dst[:, mt, sl], ps[:])

        # ---- phase 2: attention ---------------------------------------
        attnT = persist.tile([128, MT, S], f32r)
        with tc.tile_pool(name="p2sc", bufs=3, space="PSUM") as p2sc, \
             tc.tile_pool(name="p2at", bufs=3, space="PSUM") as p2at, \
             tc.tile_pool(name="p2rb", bufs=2, space="PSUM") as p2rb, \
             tc.tile_pool(name="p2sb", bufs=4) as p2sb, \
             tc.tile_pool(name="p2r", bufs=4) as p2r:
            for mt in range(MT):
                for c2 in range(NC2):
                    qsl = slice(c2 * 512, (c2 + 1) * 512)
                    for hh in range(2):          # two heads of this row-tile
                        ph = hh * 64
                        h = mt * 2 + hh          # local head index
                        at = p2at.tile([HEAD_DIM + 1, 512], f32, tag="at")
                        for kt in range(ST):
                            sc = p2sc.tile([128, 512], f32, tag="sc")
                            nc.tensor.matmul(
                                out=sc[:],
                                lhsT=kT[ph:ph + 64, mt, kt * 128:(kt + 1) * 128],
                                rhs=qT[ph:ph + 64, mt, qsl],
                                start=True, stop=True,
                                tile_position=(ph, 0))
                            pt = p2sb.tile([128, 512], f32r, tag="pt")
                            nc.scalar.activation(
                                pt[:], sc[:], AF.Exp,
                                bias=maskb[:, kt:kt + 1], scale=0.125)
                            nc.tensor.matmul(
                                out=at[:], lhsT=vaug[:, kt, h, :], rhs=pt[:],
                                start=(kt == 0), stop=(kt == ST - 1))
                        rec = p2r.tile([1, 512], f32r, tag="rec")
                        with nc.allow_low_precision(
                                reason="float32r feed for PE broadcast"):
                            nc.vector.reciprocal(
                                rec[:], at[HEAD_DIM:HEAD_DIM + 1, :])
                        rb = p2rb.tile([HEAD_DIM, 512], f32, tag="rb")
                        nc.tensor.matmul(out=rb[:], lhsT=ones[0:1, 0:HEAD_DIM],
                                         rhs=rec[:], start=True, stop=True)
                        rbs = p2r.tile([HEAD_DIM, 512], f32, tag="rbs")
                        nc.vector.tensor_copy(rbs[:], rb[:])
                        nc.vector.tensor_tensor(
                            attnT[ph:ph + 64, mt, qsl],
                            at[0:HEAD_DIM, :], rbs[:], op=OP.mult)

        # ---- phase 3: output projection -------------------------------
        with tc.tile_pool(name="p3ps", bufs=4, space="PSUM") as p3ps, \
             tc.tile_pool(name="p3sb", bufs=3) as p3sb:
            for qt in range(ST):
                ob = p3sb.tile([128, DIM], f32, tag="ob")
                for c2 in range(DIM // 512):
                    nsl = slice(c2 * 512, (c2 + 1) * 512)
                    ps = p3ps.tile([128, 512], f32, tag="ps3")
                    nc.tensor.matmul(
                        out=ps[:], lhsT=ones[0:1, 0:128], rhs=bo[0:1, nsl],
                        start=True, stop=False)
                    for mt in range(MT):
                        nc.tensor.matmul(
                            out=ps[:],
                            lhsT=attnT[:, mt, qt * 128:(qt + 1) * 128],
                            rhs=wo[:, mt, nsl],
                            start=False, stop=(mt == MT - 1))
                    nc.vector.tensor_scalar(
                        ob[:, nsl], ps[:], mask01[:, qt:qt + 1], None,
                        op0=OP.mult)
                nc.sync.dma_start(out=out_d.ap()[qt * 128:(qt + 1) * 128, :],
                                  in_=ob[:])

    nc.compile()
    return nc


def _get_nc():
    if "nc" not in _CACHE:
        _CACHE["nc"] = _build()
    return _CACHE["nc"]


def _prep_inputs(x, mask, freqs, Wq, bq, Wk, bk, Wv, bv, Wo, bo):
    f = np.asarray(freqs, np.float32)[0]              # [S, HEAD_DIM]
    # reference rotates only the first rot_dim=64 channels of the FLAT
    # inner dim -> rows 0-63 of row-tile 0 on the hg=0 core; everything
    # else is identity (cos=1, sin=0).
    cos2 = np.ones((128, S), np.float32)
    sin2 = np.zeros((128, S), np.float32)
    cos2[0:HEAD_DIM] = np.cos(f.T)
    sin2[0:HEAD_DIM] = np.sin(f.T)
    ident = np.ones((128, S), np.float32)
    identz = np.zeros((128, S), np.float32)

    prt = np.zeros((128, 128), np.float32)            # P_rot^T
    i = np.arange(0, 128, 2)
    prt[i + 1, i] = -1.0                              # P_rot[2i, 2i+1] = -1
    prt[i, i + 1] = 1.0                               # P_rot[2i+1, 2i] = +1

    def lhsT_w(w):                                    # [DIM, DSH] -> lhsT tiles
        return np.ascontiguousarray(
            w.reshape(KT, 128, MT, 128).transpose(1, 0, 2, 3), np.float32)

    in_maps = []
    for b in range(B):
        xT = np.ascontiguousarray(
            np.asarray(x[b], np.float32).T.reshape(KT, 128, S)
            .transpose(1, 0, 2))
        m = np.asarray(mask[b])
        maskb = np.ascontiguousarray(
            np.where(m, 0.0, MASK_NEG).astype(np.float32).reshape(ST, 128).T)
        mask01 = np.ascontiguousarray(
            m.astype(np.float32).reshape(ST, 128).T)
        for hg in range(HG):
            dsl = slice(hg * DSH, (hg + 1) * DSH)
            in_maps.append({
                "xT": xT,
                "wq": lhsT_w(np.asarray(Wq, np.float32)[:, dsl]),
                "wk": lhsT_w(np.asarray(Wk, np.float32)[:, dsl]),
                "wv": np.ascontiguousarray(
                    np.asarray(Wv, np.float32)[:, dsl]
                    .reshape(KT, 128, DSH).transpose(1, 0, 2)),
                "wo": np.ascontiguousarray(
                    np.asarray(Wo, np.float32)[dsl, :]
                    .reshape(MT, 128, DIM).transpose(1, 0, 2)),
                "bq": np.asarray(bq, np.float32)[None, dsl].copy(),
                "bk": np.asarray(bk, np.float32)[None, dsl].copy(),
                "bv": np.asarray(bv, np.float32)[None, dsl].copy(),
                "bo": (np.asarray(bo, np.float32) * 0.5)[None, :].copy(),
                "cos2": cos2 if hg == 0 else ident,
                "sin2": sin2 if hg == 0 else identz,
                "prt": prt,
                "maskb": maskb, "mask01": mask01,
            })
    return in_maps


def run(trace=False, **inputs):
    from concourse import bass_utils
    if trace:
        _install_ntff_hook()
    nc = _get_nc()
    in_maps = _prep_inputs(**inputs)
    res = bass_utils.run_bass_kernel_spmd(
        nc, in_maps, core_ids=list(range(NCORES)), trace=trace)
    out = np.empty((B, S, DIM), np.float32)
    for b in range(B):
        out[b] = res.results[2 * b]["out"] + res.results[2 * b + 1]["out"]
    return out, res


def kernel(**inputs):
    out, _ = run(trace=False, **inputs)
    return out


def _install_ntff_hook():
    """Register the axon NTFF profiling hook missing from the antenv stub."""
    import sys, types
    try:
        import antenv.axon_hooks  # noqa: F401
        return
    except ImportError:
        pass
    from trn_agent_boot.trn_boot import _ntff_profile_via_ctypes
    hook = _ntff_profile_via_ctypes('/opt/axon/libaxon_pjrt.so')
    mod = types.ModuleType('antenv.axon_hooks')
    mod.get_axon_ntff_profile_hook = lambda: hook
    mod.set_axon_ntff_profile_hook = lambda h: None
    sys.modules['antenv.axon_hooks'] = mod


# revision 18
# speedup vs baseline: 1.0632x; 1.0632x over previous
"""Trainium2 Bass kernel for nn_Attention (B=4, S=1024, DIM=1024, H=16, Dh=64).

Sharding: 8 cores = 4 batches x 2 head-groups (8 heads / 512 inner channels
each).  Each core computes q/k/v projections for its head shard, RoPE,
attention, and a partial output projection (its rows of Wo); the host sums
the two head-group partials per batch (the tensor-parallel all-reduce done
on host) and concatenates batches.

Device dataflow (per core), matmul operands in fp16 (fp32 PSUM accumulate):
  x^T staged in SBUF ->
  Q^T,K^T = W^T @ x^T      (bias added on the PSUM->SBUF pass)
  RoPE on the first 64 flat channels only (reference rotates rot_dim=64 of
  the flat inner dim): qr = (q+b)*cos + P_rot@((q+b)*sin), P_rot on PE.
  scores^T[k,q] = K_h @ Q_h^T   (K=64; the two heads of a row-tile issue
                                 back-to-back on row groups 0/64 -> concurrent)
  P^T = exp(scores^T/8 + maskbias[k])  (ACT, one op per head over q=1024;
                                        key mask folded into the exp bias)
  attn^T[c,q] (+rowsum via a ones-column in V_aug) = V_aug^T @ P^T
  rowsums gathered -> one batched reciprocal -> DMA partition-broadcast ->
  normalize -> out[q,:] = attn^T.T @ Wo_shard + bo/2 (K=1 matmul), masked
  rows zeroed on the PSUM->SBUF copy.
"""

import numpy as np

B, S, DIM, HEADS, HEAD_DIM = 4, 1024, 1024, 16, 64
INNER = HEADS * HEAD_DIM
HG = 2                      # head groups (tensor-parallel shards)
DSH = INNER // HG           # 512 inner channels per core
HSH = HEADS // HG           # 8 heads per core
NCORES = B * HG
KT = DIM // 128             # 8 contraction tiles
MT = DSH // 128             # 4 output row tiles for Q^T/K^T
ST = S // 128               # 8 seq tiles
MASK_NEG = -80.0

_CACHE = {}


def _build():
    import concourse.tile as tile
    from concourse import bacc, mybir

    f32 = mybir.dt.float32
    f16 = mybir.dt.float16
    AF = mybir.ActivationFunctionType
    OP = mybir.AluOpType

    nc = bacc.Bacc("TRN2", target_bir_lowering=False, debug=False)

    xT_d = nc.dram_tensor("xT", [128, KT, S], f16, kind="ExternalInput")
    wq_d = nc.dram_tensor("wq", [128, KT, MT, 128], f16, kind="ExternalInput")
    wk_d = nc.dram_tensor("wk", [128, KT, MT, 128], f16, kind="ExternalInput")
    wv_d = nc.dram_tensor("wv", [128, KT, DSH], f16, kind="ExternalInput")
    wo_d = nc.dram_tensor("wo", [128, MT, DIM], f16, kind="ExternalInput")
    bq_d = nc.dram_tensor("bq", [128, MT], f32, kind="ExternalInput")
    bk_d = nc.dram_tensor("bk", [128, MT], f32, kind="ExternalInput")
    bv_d = nc.dram_tensor("bv", [128, DSH], f32, kind="ExternalInput")
    bo_d = nc.dram_tensor("bo", [1, DIM], f16, kind="ExternalInput")
    cos_d = nc.dram_tensor("cos2", [128, S], f32, kind="ExternalInput")
    sin_d = nc.dram_tensor("sin2", [128, S], f32, kind="ExternalInput")
    prt_d = nc.dram_tensor("prt", [128, 128], f16, kind="ExternalInput")
    maskb_d = nc.dram_tensor("maskb", [128, ST], f32, kind="ExternalInput")
    mask01_d = nc.dram_tensor("mask01", [128, ST], f32, kind="ExternalInput")
    out_d = nc.dram_tensor("out", [S, DIM], f32, kind="ExternalOutput")

    with tile.TileContext(nc) as tc, \
         tc.tile_pool(name="persist", bufs=1) as persist:
        with tc.tile_pool(name="w1", bufs=1) as w1:
            # phase-1-only constants
            xT = w1.tile([128, KT, S], f16)
            wq = w1.tile([128, KT, MT, 128], f16)
            wk = w1.tile([128, KT, MT, 128], f16)
            wv = w1.tile([128, KT, DSH], f16)
            bq = w1.tile([128, MT], f32)
            bk = w1.tile([128, MT], f32)
            bv = w1.tile([128, DSH], f32)
            cos2 = w1.tile([128, S], f32)
            sin2 = w1.tile([128, S], f32)
            prt = w1.tile([128, 128], f16)
            for kt in range(KT):            # split so first matmuls start early
                nc.sync.dma_start(out=xT[:, kt, :], in_=xT_d.ap()[:, kt, :])
                nc.sync.dma_start(out=wq[:, kt], in_=wq_d.ap()[:, kt])
                nc.sync.dma_start(out=wk[:, kt], in_=wk_d.ap()[:, kt])
                nc.sync.dma_start(out=wv[:, kt], in_=wv_d.ap()[:, kt])
            for t, d in [(bq, bq_d), (bk, bk_d), (bv, bv_d),
                         (cos2, cos_d), (sin2, sin_d), (prt, prt_d)]:
                nc.sync.dma_start(out=t[:], in_=d.ap())
            # persistent across phases
            wo = persist.tile([128, MT, DIM], f16)
            bo = persist.tile([1, DIM], f16)
            maskb = persist.tile([128, ST], f32)
            mask01 = persist.tile([128, ST], f32)
            ones = persist.tile([1, S], f16)
            for t, d in [(wo, wo_d), (bo, bo_d), (maskb, maskb_d),
                         (mask01, mask01_d)]:
                nc.sync.dma_start(out=t[:], in_=d.ap())
            ones_f = w1.tile([128, S], f32)
            nc.vector.memset(ones_f[:], 1.0)
            nc.vector.tensor_copy(ones[:], ones_f[0:1, :])

            qT = persist.tile([128, MT, S], f16)
            kT = persist.tile([128, MT, S], f16)
            vv = persist.tile([128, ST, HSH, HEAD_DIM], f16)
            ones_col = persist.tile([128, 1], f16)
            nc.vector.tensor_copy(ones_col[:], ones_f[:, 0:1])

            # ---- phase 1: projections + RoPE -------------------------
            with tc.tile_pool(name="p1ps", bufs=4, space="PSUM") as p1ps, \
                 tc.tile_pool(name="p1pp", bufs=2, space="PSUM") as p1pp, \
                 tc.tile_pool(name="p1sb", bufs=3) as p1sb:
                # V first (PV needs all of it)
                for st in range(ST):
                    ps = p1ps.tile([128, DSH], f32, tag="ps")
                    for kt in range(KT):
                        nc.tensor.matmul(
                            out=ps[:],
                            lhsT=xT[:, kt, st * 128:(st + 1) * 128],
                            rhs=wv[:, kt, :],
                            start=(kt == 0), stop=(kt == KT - 1))
                    nc.vector.tensor_tensor(
                        vv[:, st, :, :],
                        ps[:].rearrange("p (h d) -> p h d", h=HSH),
                        bv[:].rearrange("p (h d) -> p h d", h=HSH),
                        op=OP.add)

                # K then Q per row-tile, in 512-wide column chunks
                for dst, w, b in ((kT, wk, bk), (qT, wq, bq)):
                    for mt in range(MT):
                        for c2 in range(2):
                            sl = slice(c2 * 512, (c2 + 1) * 512)
                            ps = p1ps.tile([128, 512], f32, tag="ps")
                            for kt in range(KT):
                                nc.tensor.matmul(
                                    out=ps[:],
                                    lhsT=w[:, kt, mt, :],
                                    rhs=xT[:, kt, sl],
                                    start=(kt == 0), stop=(kt == KT - 1))
                            if mt == 0:
                                # only the first 64 flat channels are RoPE'd;
                                # rows 64-127 (and the hg=1 core entirely)
                                # get identity via cos=1/sin=0 from the host.
                                sinp = p1sb.tile([128, 512], f16, tag="sinp")
                                nc.vector.scalar_tensor_tensor(
                                    sinp[:], ps[:], b[:, mt:mt + 1],
                                    sin2[:, sl], op0=OP.add, op1=OP.mult)
                                cosp = p1sb.tile([128, 512], f32, tag="cosp")
                                nc.vector.scalar_tensor_tensor(
                                    cosp[:], ps[:], b[:, mt:mt + 1],
                                    cos2[:, sl], op0=OP.add, op1=OP.mult)
                                pp = p1pp.tile([128, 512], f32, tag="pp")
                                nc.tensor.matmul(out=pp[:], lhsT=prt[:],
                                                 rhs=sinp[:],
                                                 start=True, stop=True)
                                nc.vector.tensor_tensor(
                                    dst[:, mt, sl], cosp[:], pp[:], op=OP.add)
                            else:
                                nc.vector.tensor_scalar(
                                    dst[:, mt, sl], ps[:], b[:, mt:mt + 1],
                                    None, op0=OP.add)

        # ---- phase 2: attention ---------------------------------------
        attU = persist.tile([128, MT, S], f16)
        # rowsums live at partitions 0/32/64/96 (col-group constraint);
        # row 32*(hh*2+c2), col block mt = rowsum of head 2mt+hh, q-chunk c2
        rssum = persist.tile([97, MT, 512], f32)
        with tc.tile_pool(name="p2sc", bufs=1, space="PSUM") as p2sc, \
             tc.tile_pool(name="p2at", bufs=1, space="PSUM") as p2at, \
             tc.tile_pool(name="p2sb", bufs=2) as p2sb:
            for mt in range(MT):
                # both heads' unnormalized attn share one PSUM tile per
                # q-chunk (h0 -> partitions 0-63 via col group 0, h1 ->
                # 64-127 via col group 64: concurrent sub-array matmuls);
                # rowsums land in a [97, 512] tile at partitions 0/32/64/96.
                at = {c2: p2at.tile([128, 512], f32, name=f"at{c2}",
                                    tag=f"at{c2}") for c2 in range(2)}
                rsps = p2at.tile([97, 512], f32, tag="rsps")
                sch = {}
                for kt in range(ST):
                    for hh in range(2):
                        sch[hh] = p2sc.tile([128, S], f32, name=f"sc{hh}",
                                            tag=f"sc{hh}")
                    for c2 in range(2):
                        qsl = slice(c2 * 512, (c2 + 1) * 512)
                        for hh in range(2):   # adjacent pair -> concurrent
                            ph = hh * 64
                            nc.tensor.matmul(
                                out=sch[hh][:, qsl],
                                lhsT=kT[ph:ph + 64, mt,
                                        kt * 128:(kt + 1) * 128],
                                rhs=qT[ph:ph + 64, mt, qsl],
                                start=True, stop=True,
                                tile_position=(ph, 0))
                    pt = {}
                    for hh in range(2):
                        pt[hh] = p2sb.tile([128, S], f16, name=f"pt{hh}",
                                           tag=f"pt{hh}")
                        nc.scalar.activation(
                            pt[hh][:], sch[hh][:], AF.Exp,
                            bias=maskb[:, kt:kt + 1], scale=0.125)
                    first, last = (kt == 0), (kt == ST - 1)
                    for c2 in range(2):
                        qsl = slice(c2 * 512, (c2 + 1) * 512)
                        for hh in range(2):   # col groups 0 / 64: concurrent
                            nc.tensor.matmul(
                                out=at[c2][hh * 64:hh * 64 + 64, :],
                                lhsT=vv[:, kt, mt * 2 + hh, :],
                                rhs=pt[hh][:, qsl],
                                start=first, stop=last,
                                tile_position=(0, hh * 64))
                        for hh in range(2):   # rowsums, col groups 0/32/64/96
                            r = 32 * (hh * 2 + c2)
                            nc.tensor.matmul(
                                out=rsps[r:r + 1, :],
                                lhsT=ones_col[:],
                                rhs=pt[hh][:, qsl],
                                start=first, stop=last,
                                tile_position=(0, r))
                for c2 in range(2):
                    qsl = slice(c2 * 512, (c2 + 1) * 512)
                    nc.vector.tensor_copy(attU[:, mt, qsl], at[c2][:])
                    for hh in range(2):
                        r = 32 * (hh * 2 + c2)
                        nc.vector.tensor_copy(rssum[r:r + 1, mt, :],
                                              rsps[r:r + 1, :])

        # ---- phase 2b: normalize --------------------------------------
        # reciprocal in place (partitions between the four used rows hold
        # junk; their reciprocals are computed and ignored), bounce through
        # DRAM to partition-broadcast each head's 1/rowsum row.
        recq = persist.tile([97, MT, 512], f32)
        recd = nc.dram_tensor("recd", [97, MT, 512], f32)
        nc.vector.reciprocal(recq[:], rssum[:])
        nc.sync.dma_start(out=recd.ap(), in_=recq[:])
        with tc.tile_pool(name="p2r", bufs=3) as p2r:
            for h in range(HSH):
                mt, hh = h // 2, h % 2
                ph = hh * 64
                rb = p2r.tile([128, 2, 512], f32, tag="rb")
                nc.sync.dma_start(
                    out=rb[ph:ph + 64],
                    in_=recd.ap()[64 * hh:64 * hh + 33:32,
                                  mt, :].partition_broadcast(HEAD_DIM))
                nc.vector.tensor_tensor(
                    attU[ph:ph + 64, mt, :], attU[ph:ph + 64, mt, :],
                    rb[ph:ph + 64].rearrange("p a b -> p (a b)"), op=OP.mult)

        # ---- phase 3: output projection -------------------------------
        with tc.tile_pool(name="p3ps", bufs=4, space="PSUM") as p3ps, \
             tc.tile_pool(name="p3sb", bufs=3) as p3sb:
            for qt in range(ST):
                ob = p3sb.tile([128, DIM], f32, tag="ob")
                for c2 in range(DIM // 512):
                    nsl = slice(c2 * 512, (c2 + 1) * 512)
                    ps = p3ps.tile([128, 512], f32, tag="ps3")
                    nc.tensor.matmul(
                        out=ps[:], lhsT=ones[0:1, 0:128], rhs=bo[0:1, nsl],
                        start=True, stop=False)
                    for mt in range(MT):
                        nc.tensor.matmul(
                            out=ps[:],
                            lhsT=attU[:, mt, qt * 128:(qt + 1) * 128],
                            rhs=wo[:, mt, nsl],
                            start=False, stop=(mt == MT - 1))
                    nc.vector.tensor_scalar(
                        ob[:, nsl], ps[:], mask01[:, qt:qt + 1], None,
                        op0=OP.mult)
                nc.sync.dma_start(out=out_d.ap()[qt * 128:(qt + 1) * 128, :],
                                  in_=ob[:])

    nc.compile()
    return nc


def _get_nc():
    if "nc" not in _CACHE:
        _CACHE["nc"] = _build()
    return _CACHE["nc"]


def _prep_inputs(x, mask, freqs, Wq, bq, Wk, bk, Wv, bv, Wo, bo):
    f = np.asarray(freqs, np.float32)[0]              # [S, HEAD_DIM]
    # reference rotates only the first rot_dim=64 channels of the FLAT
    # inner dim -> rows 0-63 of row-tile 0 on the hg=0 core; everything
    # else is identity (cos=1, sin=0).
    cos2 = np.ones((128, S), np.float32)
    sin2 = np.zeros((128, S), np.float32)
    cos2[0:HEAD_DIM] = np.cos(f.T)
    sin2[0:HEAD_DIM] = np.sin(f.T)
    ident = np.ones((128, S), np.float32)
    identz = np.zeros((128, S), np.float32)

    prt = np.zeros((128, 128), np.float16)            # P_rot^T
    i = np.arange(0, 128, 2)
    prt[i + 1, i] = -1.0                              # P_rot[2i, 2i+1] = -1
    prt[i, i + 1] = 1.0                               # P_rot[2i+1, 2i] = +1

    def lhsT_w(w):                                    # [DIM, DSH] -> lhsT tiles
        return np.ascontiguousarray(
            w.reshape(KT, 128, MT, 128).transpose(1, 0, 2, 3)).astype(np.float16)

    def col(b):                                       # [DSH] -> [128, MT]
        return np.ascontiguousarray(b.reshape(MT, 128).T.astype(np.float32))

    in_maps = []
    for b in range(B):
        xT = np.ascontiguousarray(
            np.asarray(x[b], np.float32).T.reshape(KT, 128, S)
            .transpose(1, 0, 2)).astype(np.float16)
        m = np.asarray(mask[b])
        maskb = np.ascontiguousarray(
            np.where(m, 0.0, MASK_NEG).astype(np.float32).reshape(ST, 128).T)
        mask01 = np.ascontiguousarray(
            m.astype(np.float32).reshape(ST, 128).T)
        for hg in range(HG):
            dsl = slice(hg * DSH, (hg + 1) * DSH)
            in_maps.append({
                "xT": xT,
                "wq": lhsT_w(np.asarray(Wq, np.float32)[:, dsl]),
                "wk": lhsT_w(np.asarray(Wk, np.float32)[:, dsl]),
                "wv": np.ascontiguousarray(
                    np.asarray(Wv, np.float32)[:, dsl]
                    .reshape(KT, 128, DSH).transpose(1, 0, 2)).astype(np.float16),
                "wo": np.ascontiguousarray(
                    np.asarray(Wo, np.float32)[dsl, :]
                    .reshape(MT, 128, DIM).transpose(1, 0, 2)).astype(np.float16),
                "bq": col(np.asarray(bq, np.float32)[dsl]),
                "bk": col(np.asarray(bk, np.float32)[dsl]),
                "bv": np.broadcast_to(
                    np.asarray(bv, np.float32)[dsl], (128, DSH)).copy(),
                "bo": (np.asarray(bo, np.float32) * 0.5)[None, :]
                    .astype(np.float16).copy(),
                "cos2": cos2 if hg == 0 else ident,
                "sin2": sin2 if hg == 0 else identz,
                "prt": prt,
                "maskb": maskb, "mask01": mask01,
            })
    return in_maps


def run(trace=False, **inputs):
    from concourse import bass_utils
    if trace:
        _install_ntff_hook()
    nc = _get_nc()
    in_maps = _prep_inputs(**inputs)
    res = bass_utils.run_bass_kernel_spmd(
        nc, in_maps, core_ids=list(range(NCORES)), trace=trace)
    out = np.empty((B, S, DIM), np.float32)
    for b in range(B):
        out[b] = res.results[2 * b]["out"] + res.results[2 * b + 1]["out"]
    return out, res


def kernel(**inputs):
    out, _ = run(trace=False, **inputs)
    return out


def _install_ntff_hook():
    """Register the axon NTFF profiling hook missing from the antenv stub."""
    import sys, types
    try:
        import antenv.axon_hooks  # noqa: F401
        return
    except ImportError:
        pass
    from trn_agent_boot.trn_boot import _ntff_profile_via_ctypes
    hook = _ntff_profile_via_ctypes('/opt/axon/libaxon_pjrt.so')
    mod = types.ModuleType('antenv.axon_hooks')
    mod.get_axon_ntff_profile_hook = lambda: hook
    mod.set_axon_ntff_profile_hook = lambda h: None
    sys.modules['antenv.axon_hooks'] = mod


# revision 21
# speedup vs baseline: 1.4401x; 1.3544x over previous
"""Trainium2 Bass kernel for nn_Attention (B=4, S=1024, DIM=1024, H=16, Dh=64).

Sharding: 8 cores = 4 batches x 2 head-groups (8 heads / 512 inner channels
each).  Each core computes q/k/v projections for its head shard, RoPE,
attention, and a partial output projection (its rows of Wo); the host sums
the two head-group partials per batch (the tensor-parallel all-reduce done
on host) and concatenates batches.

Device dataflow (per core), matmul operands in fp16 (fp32 PSUM accumulate):
  x^T staged in SBUF ->
  Q^T,K^T = W^T @ x^T      (bias added on the PSUM->SBUF pass)
  RoPE on the first 64 flat channels only (reference rotates rot_dim=64 of
  the flat inner dim): qr = (q+b)*cos + P_rot@((q+b)*sin), P_rot on PE.
  scores^T[k,q] = K_h @ Q_h^T   (K=64; the two heads of a row-tile issue
                                 back-to-back on row groups 0/64 -> concurrent)
  P^T = exp(scores^T/8 + maskbias[k])  (ACT, one op per head over q=1024;
                                        key mask folded into the exp bias)
  attn^T[c,q] (+rowsum via a ones-column in V_aug) = V_aug^T @ P^T
  rowsums gathered -> one batched reciprocal -> DMA partition-broadcast ->
  normalize -> out[q,:] = attn^T.T @ Wo_shard + bo/2 (K=1 matmul), masked
  rows zeroed on the PSUM->SBUF copy.
"""

import numpy as np

B, S, DIM, HEADS, HEAD_DIM = 4, 1024, 1024, 16, 64
INNER = HEADS * HEAD_DIM
HG = 2                      # head groups (tensor-parallel shards)
DSH = INNER // HG           # 512 inner channels per core
HSH = HEADS // HG           # 8 heads per core
NCORES = B * HG
KT = DIM // 128             # 8 contraction tiles
MT = DSH // 128             # 4 output row tiles for Q^T/K^T
ST = S // 128               # 8 seq tiles
MASK_NEG = -80.0

_CACHE = {}


def _build():
    import concourse.tile as tile
    from concourse import bacc, mybir

    f32 = mybir.dt.float32
    f16 = mybir.dt.float16
    AF = mybir.ActivationFunctionType
    OP = mybir.AluOpType

    nc = bacc.Bacc("TRN2", target_bir_lowering=False, debug=False)

    xT_d = nc.dram_tensor("xT", [128, KT, S], f16, kind="ExternalInput")
    wq_d = nc.dram_tensor("wq", [128, KT, MT, 128], f16, kind="ExternalInput")
    wk_d = nc.dram_tensor("wk", [128, KT, MT, 128], f16, kind="ExternalInput")
    wv_d = nc.dram_tensor("wv", [128, KT, DSH], f16, kind="ExternalInput")
    wo_d = nc.dram_tensor("wo", [128, MT, DIM], f16, kind="ExternalInput")
    bq_d = nc.dram_tensor("bq", [128, MT], f32, kind="ExternalInput")
    bk_d = nc.dram_tensor("bk", [128, MT], f32, kind="ExternalInput")
    bv_d = nc.dram_tensor("bv", [128, DSH], f32, kind="ExternalInput")
    bo_d = nc.dram_tensor("bo", [1, DIM], f16, kind="ExternalInput")
    cos_d = nc.dram_tensor("cos2", [128, S], f32, kind="ExternalInput")
    sin_d = nc.dram_tensor("sin2", [128, S], f32, kind="ExternalInput")
    prt_d = nc.dram_tensor("prt", [128, 128], f16, kind="ExternalInput")
    maskb_d = nc.dram_tensor("maskb", [128, ST], f32, kind="ExternalInput")
    mask01_d = nc.dram_tensor("mask01", [128, ST], f32, kind="ExternalInput")
    out_d = nc.dram_tensor("out", [S, DIM], f32, kind="ExternalOutput")

    with tile.TileContext(nc) as tc, \
         tc.tile_pool(name="persist", bufs=1) as persist:
        with tc.tile_pool(name="w1", bufs=1) as w1:
            # phase-1-only constants
            xT = w1.tile([128, KT, S], f16)
            wq = w1.tile([128, KT, MT, 128], f16)
            wk = w1.tile([128, KT, MT, 128], f16)
            wv = w1.tile([128, KT, DSH], f16)
            bq = w1.tile([128, MT], f32)
            bk = w1.tile([128, MT], f32)
            bv = w1.tile([128, DSH], f32)
            cos2 = w1.tile([128, S], f32)
            sin2 = w1.tile([128, S], f32)
            prt = w1.tile([128, 128], f16)
            for kt in range(KT):            # split so first matmuls start early
                nc.sync.dma_start(out=xT[:, kt, :], in_=xT_d.ap()[:, kt, :])
            for kt in range(KT):
                nc.sync.dma_start(out=wk[:, kt], in_=wk_d.ap()[:, kt])
            for kt in range(KT):
                nc.sync.dma_start(out=wq[:, kt], in_=wq_d.ap()[:, kt])
            for kt in range(KT):
                nc.sync.dma_start(out=wv[:, kt], in_=wv_d.ap()[:, kt])
            for t, d in [(bq, bq_d), (bk, bk_d), (bv, bv_d),
                         (cos2, cos_d), (sin2, sin_d), (prt, prt_d)]:
                nc.sync.dma_start(out=t[:], in_=d.ap())
            # persistent across phases
            wo = persist.tile([128, MT, DIM], f16)
            bo = persist.tile([1, DIM], f16)
            maskb = persist.tile([128, ST], f32)
            mask01 = persist.tile([128, ST], f32)
            ones = persist.tile([1, S], f16)
            for t, d in [(wo, wo_d), (bo, bo_d), (maskb, maskb_d),
                         (mask01, mask01_d)]:
                nc.sync.dma_start(out=t[:], in_=d.ap())
            ones_f = w1.tile([128, S], f32)
            nc.vector.memset(ones_f[:], 1.0)
            nc.vector.tensor_copy(ones[:], ones_f[0:1, :])

            qT = persist.tile([128, MT, S], f16)
            kT = persist.tile([128, MT, S], f16)
            vv = persist.tile([128, ST, HSH, HEAD_DIM], f16)
            ones_col = persist.tile([128, 1], f16)
            nc.vector.tensor_copy(ones_col[:], ones_f[:, 0:1])

            # ---- phase 1: projections + RoPE -------------------------
            with tc.tile_pool(name="p1ps", bufs=4, space="PSUM") as p1ps, \
                 tc.tile_pool(name="p1pp", bufs=2, space="PSUM") as p1pp, \
                 tc.tile_pool(name="p1sb", bufs=3) as p1sb:
                def proj_kq(dst, w, b, mt):
                    for c2 in range(2):
                        sl = slice(c2 * 512, (c2 + 1) * 512)
                        ps = p1ps.tile([128, 512], f32, tag="ps", name="ps")
                        for kt in range(KT):
                            nc.tensor.matmul(
                                out=ps[:],
                                lhsT=w[:, kt, mt, :],
                                rhs=xT[:, kt, sl],
                                start=(kt == 0), stop=(kt == KT - 1))
                        if mt == 0:
                            # only the first 64 flat channels are RoPE'd;
                            # rows 64-127 (and the hg=1 core entirely)
                            # get identity via cos=1/sin=0 from the host.
                            sinp = p1sb.tile([128, 512], f16, tag="sinp",
                                             name="sinp")
                            nc.vector.scalar_tensor_tensor(
                                sinp[:], ps[:], b[:, mt:mt + 1],
                                sin2[:, sl], op0=OP.add, op1=OP.mult)
                            cosp = p1sb.tile([128, 512], f32, tag="cosp",
                                             name="cosp")
                            nc.vector.scalar_tensor_tensor(
                                cosp[:], ps[:], b[:, mt:mt + 1],
                                cos2[:, sl], op0=OP.add, op1=OP.mult)
                            pp = p1pp.tile([128, 512], f32, tag="pp",
                                           name="pp")
                            nc.tensor.matmul(out=pp[:], lhsT=prt[:],
                                             rhs=sinp[:],
                                             start=True, stop=True)
                            nc.vector.tensor_tensor(
                                dst[:, mt, sl], cosp[:], pp[:], op=OP.add)
                        else:
                            nc.vector.tensor_scalar(
                                dst[:, mt, sl], ps[:], b[:, mt:mt + 1],
                                None, op0=OP.add)

                def proj_v(st):
                    ps = p1ps.tile([128, DSH], f32, tag="ps", name="ps")
                    for kt in range(KT):
                        nc.tensor.matmul(
                            out=ps[:],
                            lhsT=xT[:, kt, st * 128:(st + 1) * 128],
                            rhs=wv[:, kt, :],
                            start=(kt == 0), stop=(kt == KT - 1))
                    nc.vector.tensor_tensor(
                        vv[:, st, :, :],
                        ps[:].rearrange("p (h d) -> p h d", h=HSH),
                        bv[:].rearrange("p (h d) -> p h d", h=HSH),
                        op=OP.add)

                # K0/Q0 first so attention on row-tile 0 unblocks early;
                # V follows in key order (PV consumes V s-tiles in order).
                proj_kq(kT, wk, bk, 0)
                proj_kq(qT, wq, bq, 0)
                for st in range(ST):
                    proj_v(st)
                for mt in range(1, MT):
                    proj_kq(kT, wk, bk, mt)
                    proj_kq(qT, wq, bq, mt)

        # ---- phase 2: attention ---------------------------------------
        attU = persist.tile([128, MT, S], f16)
        # rowsums live at partitions 0/32/64/96 (col-group constraint);
        # row 32*(hh*2+c2), col block mt = rowsum of head 2mt+hh, q-chunk c2
        rssum = persist.tile([97, MT, 512], f32)
        recq = persist.tile([97, MT, 512], f32)
        recd = nc.dram_tensor("recd", [97, MT, 512], f32)
        with tc.tile_pool(name="p2sc", bufs=1, space="PSUM") as p2sc, \
             tc.tile_pool(name="p2at", bufs=1, space="PSUM") as p2at, \
             tc.tile_pool(name="p2sb", bufs=2) as p2sb, \
             tc.tile_pool(name="p2r", bufs=2) as p2r:
            for mt in range(MT):
                # both heads' unnormalized attn share one PSUM tile per
                # q-chunk (h0 -> partitions 0-63 via col group 0, h1 ->
                # 64-127 via col group 64: concurrent sub-array matmuls);
                # rowsums land in a [97, 512] tile at partitions 0/32/64/96.
                at = {c2: p2at.tile([128, 512], f32, name=f"at{c2}",
                                    tag=f"at{c2}") for c2 in range(2)}
                rsps = p2at.tile([97, 512], f32, tag="rsps", name="rsps")
                for kt in range(ST):
                    sch = {}
                    for hh in range(2):
                        for c2 in range(2):
                            sch[hh, c2] = p2sc.tile(
                                [128, 512], f32,
                                name=f"sc{hh}{c2}", tag=f"sc{hh}{c2}")
                    for c2 in range(2):
                        qsl = slice(c2 * 512, (c2 + 1) * 512)
                        for hh in range(2):   # adjacent pair -> concurrent
                            ph = hh * 64
                            nc.tensor.matmul(
                                out=sch[hh, c2][:],
                                lhsT=kT[ph:ph + 64, mt,
                                        kt * 128:(kt + 1) * 128],
                                rhs=qT[ph:ph + 64, mt, qsl],
                                start=True, stop=True,
                                tile_position=(ph, 0))
                    pt = {}
                    for hh in range(2):
                        for c2 in range(2):
                            pt[hh, c2] = p2sb.tile(
                                [128, 512], f16,
                                name=f"pt{hh}{c2}", tag=f"pt{hh}{c2}")
                            nc.scalar.activation(
                                pt[hh, c2][:], sch[hh, c2][:], AF.Exp,
                                bias=maskb[:, kt:kt + 1], scale=0.125)
                    first, last = (kt == 0), (kt == ST - 1)
                    for c2 in range(2):
                        for hh in range(2):   # col groups 0 / 64: concurrent
                            nc.tensor.matmul(
                                out=at[c2][hh * 64:hh * 64 + 64, :],
                                lhsT=vv[:, kt, mt * 2 + hh, :],
                                rhs=pt[hh, c2][:],
                                start=first, stop=last,
                                tile_position=(0, hh * 64))
                        for hh in range(2):   # rowsums, col groups 0/32/64/96
                            r = 32 * (hh * 2 + c2)
                            nc.tensor.matmul(
                                out=rsps[r:r + 1, :],
                                lhsT=ones_col[:],
                                rhs=pt[hh, c2][:],
                                start=first, stop=last,
                                tile_position=(0, r))
                for c2 in range(2):
                    qsl = slice(c2 * 512, (c2 + 1) * 512)
                    nc.vector.tensor_copy(attU[:, mt, qsl], at[c2][:])
                    for hh in range(2):
                        r = 32 * (hh * 2 + c2)
                        nc.vector.tensor_copy(rssum[r:r + 1, mt, :],
                                              rsps[r:r + 1, :])
                # normalize this row-tile while the next one computes:
                # reciprocal (junk partitions between the four used rows are
                # computed and ignored), DRAM bounce to partition-broadcast.
                nc.vector.reciprocal(recq[:, mt, :], rssum[:, mt, :])
                nc.sync.dma_start(out=recd.ap()[:, mt, :], in_=recq[:, mt, :])
                for hh in range(2):
                    ph = hh * 64
                    rb = p2r.tile([128, 2, 512], f32, tag="rb", name="rb")
                    nc.sync.dma_start(
                        out=rb[ph:ph + 64],
                        in_=recd.ap()[64 * hh:64 * hh + 33:32,
                                      mt, :].partition_broadcast(HEAD_DIM))
                    nc.vector.tensor_tensor(
                        attU[ph:ph + 64, mt, :], attU[ph:ph + 64, mt, :],
                        rb[ph:ph + 64].rearrange("p a b -> p (a b)"),
                        op=OP.mult)

        # ---- phase 3: output projection -------------------------------
        with tc.tile_pool(name="p3ps", bufs=4, space="PSUM") as p3ps, \
             tc.tile_pool(name="p3sb", bufs=3) as p3sb:
            for qt in range(ST):
                ob = p3sb.tile([128, DIM], f32, tag="ob")
                for c2 in range(DIM // 512):
                    nsl = slice(c2 * 512, (c2 + 1) * 512)
                    ps = p3ps.tile([128, 512], f32, tag="ps3")
                    nc.tensor.matmul(
                        out=ps[:], lhsT=ones[0:1, 0:128], rhs=bo[0:1, nsl],
                        start=True, stop=False)
                    for mt in range(MT):
                        nc.tensor.matmul(
                            out=ps[:],
                            lhsT=attU[:, mt, qt * 128:(qt + 1) * 128],
                            rhs=wo[:, mt, nsl],
                            start=False, stop=(mt == MT - 1))
                    nc.vector.tensor_scalar(
                        ob[:, nsl], ps[:], mask01[:, qt:qt + 1], None,
                        op0=OP.mult)
                nc.sync.dma_start(out=out_d.ap()[qt * 128:(qt + 1) * 128, :],
                                  in_=ob[:])

    nc.compile()
    return nc


def _get_nc():
    if "nc" not in _CACHE:
        _CACHE["nc"] = _build()
    return _CACHE["nc"]


def _prep_inputs(x, mask, freqs, Wq, bq, Wk, bk, Wv, bv, Wo, bo):
    f = np.asarray(freqs, np.float32)[0]              # [S, HEAD_DIM]
    # reference rotates only the first rot_dim=64 channels of the FLAT
    # inner dim -> rows 0-63 of row-tile 0 on the hg=0 core; everything
    # else is identity (cos=1, sin=0).
    cos2 = np.ones((128, S), np.float32)
    sin2 = np.zeros((128, S), np.float32)
    cos2[0:HEAD_DIM] = np.cos(f.T)
    sin2[0:HEAD_DIM] = np.sin(f.T)
    ident = np.ones((128, S), np.float32)
    identz = np.zeros((128, S), np.float32)

    prt = np.zeros((128, 128), np.float16)            # P_rot^T
    i = np.arange(0, 128, 2)
    prt[i + 1, i] = -1.0                              # P_rot[2i, 2i+1] = -1
    prt[i, i + 1] = 1.0                               # P_rot[2i+1, 2i] = +1

    def lhsT_w(w):                                    # [DIM, DSH] -> lhsT tiles
        return np.ascontiguousarray(
            w.reshape(KT, 128, MT, 128).transpose(1, 0, 2, 3)).astype(np.float16)

    def col(b):                                       # [DSH] -> [128, MT]
        return np.ascontiguousarray(b.reshape(MT, 128).T.astype(np.float32))

    in_maps = []
    for b in range(B):
        xT = np.ascontiguousarray(
            np.asarray(x[b], np.float32).T.reshape(KT, 128, S)
            .transpose(1, 0, 2)).astype(np.float16)
        m = np.asarray(mask[b])
        maskb = np.ascontiguousarray(
            np.where(m, 0.0, MASK_NEG).astype(np.float32).reshape(ST, 128).T)
        mask01 = np.ascontiguousarray(
            m.astype(np.float32).reshape(ST, 128).T)
        for hg in range(HG):
            dsl = slice(hg * DSH, (hg + 1) * DSH)
            in_maps.append({
                "xT": xT,
                "wq": lhsT_w(np.asarray(Wq, np.float32)[:, dsl]),
                "wk": lhsT_w(np.asarray(Wk, np.float32)[:, dsl]),
                "wv": np.ascontiguousarray(
                    np.asarray(Wv, np.float32)[:, dsl]
                    .reshape(KT, 128, DSH).transpose(1, 0, 2)).astype(np.float16),
                "wo": np.ascontiguousarray(
                    np.asarray(Wo, np.float32)[dsl, :]
                    .reshape(MT, 128, DIM).transpose(1, 0, 2)).astype(np.float16),
                "bq": col(np.asarray(bq, np.float32)[dsl]),
                "bk": col(np.asarray(bk, np.float32)[dsl]),
                "bv": np.broadcast_to(
                    np.asarray(bv, np.float32)[dsl], (128, DSH)).copy(),
                "bo": (np.asarray(bo, np.float32) * 0.5)[None, :]
                    .astype(np.float16).copy(),
                "cos2": cos2 if hg == 0 else ident,
                "sin2": sin2 if hg == 0 else identz,
                "prt": prt,
                "maskb": maskb, "mask01": mask01,
            })
    return in_maps


def run(trace=False, **inputs):
    from concourse import bass_utils
    if trace:
        _install_ntff_hook()
    nc = _get_nc()
    in_maps = _prep_inputs(**inputs)
    res = bass_utils.run_bass_kernel_spmd(
        nc, in_maps, core_ids=list(range(NCORES)), trace=trace)
    out = np.empty((B, S, DIM), np.float32)
    for b in range(B):
        out[b] = res.results[2 * b]["out"] + res.results[2 * b + 1]["out"]
    return out, res


def kernel(**inputs):
    out, _ = run(trace=False, **inputs)
    return out


def _install_ntff_hook():
    """Register the axon NTFF profiling hook missing from the antenv stub."""
    import sys, types
    try:
        import antenv.axon_hooks  # noqa: F401
        return
    except ImportError:
        pass
    from trn_agent_boot.trn_boot import _ntff_profile_via_ctypes
    hook = _ntff_profile_via_ctypes('/opt/axon/libaxon_pjrt.so')
    mod = types.ModuleType('antenv.axon_hooks')
    mod.get_axon_ntff_profile_hook = lambda: hook
    mod.set_axon_ntff_profile_hook = lambda h: None
    sys.modules['antenv.axon_hooks'] = mod


# revision 24
# speedup vs baseline: 1.6263x; 1.1293x over previous
"""Trainium2 Bass kernel for nn_Attention (B=4, S=1024, DIM=1024, H=16, Dh=64).

Sharding: 8 cores = 4 batches x 2 head-groups (8 heads / 512 inner channels
each).  Each core computes q/k/v projections for its head shard, RoPE,
attention, and a partial output projection (its rows of Wo); the host sums
the two head-group partials per batch (the tensor-parallel all-reduce done
on host) and concatenates batches.

Device dataflow (per core), matmul operands in fp16 (fp32 PSUM accumulate):
  x^T staged in SBUF ->
  Q^T,K^T = W^T @ x^T      (bias added on the PSUM->SBUF pass)
  RoPE on the first 64 flat channels only (reference rotates rot_dim=64 of
  the flat inner dim): qr = (q+b)*cos + P_rot@((q+b)*sin), P_rot on PE.
  scores^T[k,q] = K_h @ Q_h^T   (K=64; the two heads of a row-tile issue
                                 back-to-back on row groups 0/64 -> concurrent)
  P^T = exp(scores^T/8 + maskbias[k])  (ACT, one op per head over q=1024;
                                        key mask folded into the exp bias)
  attn^T[c,q] (+rowsum via a ones-column in V_aug) = V_aug^T @ P^T
  rowsums gathered -> one batched reciprocal -> DMA partition-broadcast ->
  normalize -> out[q,:] = attn^T.T @ Wo_shard + bo/2 (K=1 matmul), masked
  rows zeroed on the PSUM->SBUF copy.
"""

import numpy as np

B, S, DIM, HEADS, HEAD_DIM = 4, 1024, 1024, 16, 64
INNER = HEADS * HEAD_DIM
HG = 2                      # head groups (tensor-parallel shards)
DSH = INNER // HG           # 512 inner channels per core
HSH = HEADS // HG           # 8 heads per core
NCORES = B * HG
KT = DIM // 128             # 8 contraction tiles
MT = DSH // 128             # 4 output row tiles for Q^T/K^T
ST = S // 128               # 8 seq tiles
MASK_NEG = -80.0

_CACHE = {}


def _build():
    import concourse.tile as tile
    from concourse import bacc, mybir

    f32 = mybir.dt.float32
    f16 = mybir.dt.float16
    AF = mybir.ActivationFunctionType
    OP = mybir.AluOpType

    nc = bacc.Bacc("TRN2", target_bir_lowering=False, debug=False)

    xT_d = nc.dram_tensor("xT", [128, KT, S], f16, kind="ExternalInput")
    wq_d = nc.dram_tensor("wq", [128, KT, MT, 128], f16, kind="ExternalInput")
    wk_d = nc.dram_tensor("wk", [128, KT, MT, 128], f16, kind="ExternalInput")
    wv_d = nc.dram_tensor("wv", [128, KT, DSH], f16, kind="ExternalInput")
    wo_d = nc.dram_tensor("wo", [128, MT, DIM], f16, kind="ExternalInput")
    bq_d = nc.dram_tensor("bq", [128, MT], f32, kind="ExternalInput")
    bk_d = nc.dram_tensor("bk", [128, MT], f32, kind="ExternalInput")
    bv_d = nc.dram_tensor("bv", [128, DSH], f32, kind="ExternalInput")
    bo_d = nc.dram_tensor("bo", [1, DIM], f16, kind="ExternalInput")
    cos_d = nc.dram_tensor("cos2", [128, S], f32, kind="ExternalInput")
    sin_d = nc.dram_tensor("sin2", [128, S], f32, kind="ExternalInput")
    prt_d = nc.dram_tensor("prt", [128, 128], f16, kind="ExternalInput")
    maskb_d = nc.dram_tensor("maskb", [128, ST], f32, kind="ExternalInput")
    mask01_d = nc.dram_tensor("mask01", [128, ST], f32, kind="ExternalInput")
    out_d = nc.dram_tensor("out", [S, DIM], f32, kind="ExternalOutput")

    with tile.TileContext(nc) as tc, \
         tc.tile_pool(name="persist", bufs=1) as persist:
        with tc.tile_pool(name="w1", bufs=1) as w1:
            # phase-1-only constants
            xT = w1.tile([128, KT, S], f16)
            wq = w1.tile([128, KT, MT, 128], f16)
            wk = w1.tile([128, KT, MT, 128], f16)
            wv = w1.tile([128, KT, DSH], f16)
            bq = w1.tile([128, MT], f32)
            bk = w1.tile([128, MT], f32)
            bv = w1.tile([128, DSH], f32)
            cos2 = w1.tile([128, S], f32)
            sin2 = w1.tile([128, S], f32)
            prt = w1.tile([128, 128], f16)
            # big per-tensor DMAs (each spreads over all 16 SDMA engines),
            # issued from different engine queues so they don't serialize
            # on one HWDGE FIFO.
            nc.sync.dma_start(out=xT[:, 0:4], in_=xT_d.ap()[:, 0:4])
            nc.sync.dma_start(out=xT[:, 4:8], in_=xT_d.ap()[:, 4:8])
            nc.scalar.dma_start(out=wk[:], in_=wk_d.ap())
            nc.sync.dma_start(out=wq[:], in_=wq_d.ap())
            nc.scalar.dma_start(out=wv[:], in_=wv_d.ap())
            for t, d in [(bq, bq_d), (bk, bk_d), (bv, bv_d),
                         (cos2, cos_d), (sin2, sin_d), (prt, prt_d)]:
                nc.gpsimd.dma_start(out=t[:], in_=d.ap())
            # persistent across phases
            wo = persist.tile([128, MT, DIM], f16)
            bo = persist.tile([1, DIM], f16)
            maskb = persist.tile([128, ST], f32)
            mask01 = persist.tile([128, ST], f32)
            ones = persist.tile([1, S], f16)
            nc.scalar.dma_start(out=wo[:], in_=wo_d.ap())
            for t, d in [(bo, bo_d), (maskb, maskb_d), (mask01, mask01_d)]:
                nc.gpsimd.dma_start(out=t[:], in_=d.ap())
            ones_f = w1.tile([128, S], f32)
            nc.vector.memset(ones_f[:], 1.0)
            nc.vector.tensor_copy(ones[:], ones_f[0:1, :])

            qT = persist.tile([128, MT, S], f16)
            kT = persist.tile([128, MT, S], f16)
            vv = persist.tile([128, ST, HSH, HEAD_DIM], f16)
            ones_col = persist.tile([128, 1], f16)
            nc.vector.tensor_copy(ones_col[:], ones_f[:, 0:1])
            ones4 = persist.tile([97, HEAD_DIM], f16)
            nc.vector.tensor_copy(ones4[:], ones_f[0:97, 0:HEAD_DIM])

            # ---- phase 1: projections + RoPE -------------------------
            with tc.tile_pool(name="p1ps", bufs=4, space="PSUM") as p1ps, \
                 tc.tile_pool(name="p1pp", bufs=2, space="PSUM") as p1pp, \
                 tc.tile_pool(name="p1sb", bufs=3) as p1sb:
                def proj_kq(dst, w, b, mt):
                    for c2 in range(2):
                        sl = slice(c2 * 512, (c2 + 1) * 512)
                        ps = p1ps.tile([128, 512], f32, tag="ps", name="ps")
                        for kt in range(KT):
                            nc.tensor.matmul(
                                out=ps[:],
                                lhsT=w[:, kt, mt, :],
                                rhs=xT[:, kt, sl],
                                start=(kt == 0), stop=(kt == KT - 1))
                        if mt == 0:
                            # only the first 64 flat channels are RoPE'd;
                            # rows 64-127 (and the hg=1 core entirely)
                            # get identity via cos=1/sin=0 from the host.
                            sinp = p1sb.tile([128, 512], f16, tag="sinp",
                                             name="sinp")
                            nc.vector.scalar_tensor_tensor(
                                sinp[:], ps[:], b[:, mt:mt + 1],
                                sin2[:, sl], op0=OP.add, op1=OP.mult)
                            cosp = p1sb.tile([128, 512], f32, tag="cosp",
                                             name="cosp")
                            nc.vector.scalar_tensor_tensor(
                                cosp[:], ps[:], b[:, mt:mt + 1],
                                cos2[:, sl], op0=OP.add, op1=OP.mult)
                            pp = p1pp.tile([128, 512], f32, tag="pp",
                                           name="pp")
                            nc.tensor.matmul(out=pp[:], lhsT=prt[:],
                                             rhs=sinp[:],
                                             start=True, stop=True)
                            nc.vector.tensor_tensor(
                                dst[:, mt, sl], cosp[:], pp[:], op=OP.add)
                        else:
                            nc.vector.tensor_scalar(
                                dst[:, mt, sl], ps[:], b[:, mt:mt + 1],
                                None, op0=OP.add)

                def proj_v(st):
                    ps = p1ps.tile([128, DSH], f32, tag="ps", name="ps")
                    for kt in range(KT):
                        nc.tensor.matmul(
                            out=ps[:],
                            lhsT=xT[:, kt, st * 128:(st + 1) * 128],
                            rhs=wv[:, kt, :],
                            start=(kt == 0), stop=(kt == KT - 1))
                    nc.vector.tensor_tensor(
                        vv[:, st, :, :],
                        ps[:].rearrange("p (h d) -> p h d", h=HSH),
                        bv[:].rearrange("p (h d) -> p h d", h=HSH),
                        op=OP.add)

                # K0/Q0 first so attention on row-tile 0 unblocks early;
                # V follows in key order (PV consumes V s-tiles in order).
                proj_kq(kT, wk, bk, 0)
                proj_kq(qT, wq, bq, 0)
                for st in range(ST):
                    proj_v(st)
                for mt in range(1, MT):
                    proj_kq(kT, wk, bk, mt)
                    proj_kq(qT, wq, bq, mt)

        # ---- phase 2: attention ---------------------------------------
        attU = persist.tile([128, MT, S], f16)
        # rowsums live at partitions 0/32/64/96 (col-group constraint);
        # row 32*(hh*2+c2), col block mt = rowsum of head 2mt+hh, q-chunk c2
        rssum = persist.tile([97, MT, 512], f32)
        recq = persist.tile([97, MT, 512], f16)
        with tc.tile_pool(name="p2sc", bufs=1, space="PSUM") as p2sc, \
             tc.tile_pool(name="p2at", bufs=1, space="PSUM") as p2at, \
             tc.tile_pool(name="p2rb", bufs=1, space="PSUM") as p2rb, \
             tc.tile_pool(name="p2sb", bufs=2) as p2sb:

            def normalize(mt):
                # 1/rowsum (junk partitions between the four used rows are
                # computed and ignored), then PE-broadcast each head's row
                # across 64 partitions and scale attU in place.
                with nc.allow_low_precision(reason="fp16 recip feeds PE"):
                    nc.vector.reciprocal(recq[:, mt, :], rssum[:, mt, :])
                for hh in range(2):
                    ph = hh * 64
                    for c2 in range(2):
                        r = 32 * (hh * 2 + c2)
                        qsl = slice(c2 * 512, (c2 + 1) * 512)
                        rbps = p2rb.tile([HEAD_DIM, 512], f32, tag="rbps",
                                         name="rbps")
                        nc.tensor.matmul(
                            out=rbps[:], lhsT=ones4[r:r + 1, :],
                            rhs=recq[r:r + 1, mt, :],
                            start=True, stop=True, tile_position=(r, 0))
                        nc.vector.tensor_tensor(
                            attU[ph:ph + 64, mt, qsl],
                            attU[ph:ph + 64, mt, qsl],
                            rbps[:], op=OP.mult)

            for mt in range(MT):
                # both heads' unnormalized attn share one PSUM tile per
                # q-chunk (h0 -> partitions 0-63 via col group 0, h1 ->
                # 64-127 via col group 64: concurrent sub-array matmuls);
                # rowsums land in a [97, 512] tile at partitions 0/32/64/96.
                at = {c2: p2at.tile([128, 512], f32, name=f"at{c2}",
                                    tag=f"at{c2}") for c2 in range(2)}
                rsps = p2at.tile([97, 512], f32, tag="rsps", name="rsps")
                for kt in range(ST):
                    sch = {}
                    for hh in range(2):
                        sch[hh] = p2sc.tile([128, S], f32, name=f"sc{hh}",
                                            tag=f"sc{hh}")
                    for c2 in range(2):
                        qsl = slice(c2 * 512, (c2 + 1) * 512)
                        for hh in range(2):   # adjacent pair -> concurrent
                            ph = hh * 64
                            nc.tensor.matmul(
                                out=sch[hh][:, qsl],
                                lhsT=kT[ph:ph + 64, mt,
                                        kt * 128:(kt + 1) * 128],
                                rhs=qT[ph:ph + 64, mt, qsl],
                                start=True, stop=True,
                                tile_position=(ph, 0))
                    pt = {}
                    for hh in range(2):
                        pt[hh] = p2sb.tile([128, S], f16, name=f"pt{hh}",
                                           tag=f"pt{hh}")
                        nc.scalar.activation(
                            pt[hh][:], sch[hh][:], AF.Exp,
                            bias=maskb[:, kt:kt + 1], scale=0.125)
                    first, last = (kt == 0), (kt == ST - 1)
                    for c2 in range(2):
                        qsl = slice(c2 * 512, (c2 + 1) * 512)
                        for hh in range(2):   # col groups 0 / 64: concurrent
                            nc.tensor.matmul(
                                out=at[c2][hh * 64:hh * 64 + 64, :],
                                lhsT=vv[:, kt, mt * 2 + hh, :],
                                rhs=pt[hh][:, qsl],
                                start=first, stop=last,
                                tile_position=(0, hh * 64))
                        for hh in range(2):   # rowsums, col groups 0/32/64/96
                            r = 32 * (hh * 2 + c2)
                            nc.tensor.matmul(
                                out=rsps[r:r + 1, :],
                                lhsT=ones_col[:],
                                rhs=pt[hh][:, qsl],
                                start=first, stop=last,
                                tile_position=(0, r))
                    if kt == 3 and mt > 0:
                        # previous row-tile's normalize, emitted mid-loop so
                        # its PE ops don't head-of-line-block this row-tile's
                        # score matmuls while the reciprocal runs.
                        normalize(mt - 1)
                for c2 in range(2):
                    qsl = slice(c2 * 512, (c2 + 1) * 512)
                    nc.vector.tensor_copy(attU[:, mt, qsl], at[c2][:])
                    for hh in range(2):
                        r = 32 * (hh * 2 + c2)
                        nc.vector.tensor_copy(rssum[r:r + 1, mt, :],
                                              rsps[r:r + 1, :])
            normalize(MT - 1)

        # ---- phase 3: output projection -------------------------------
        with tc.tile_pool(name="p3ps", bufs=4, space="PSUM") as p3ps, \
             tc.tile_pool(name="p3sb", bufs=3) as p3sb:
            for qt in range(ST):
                ob = p3sb.tile([128, DIM], f32, tag="ob")
                for c2 in range(DIM // 512):
                    nsl = slice(c2 * 512, (c2 + 1) * 512)
                    ps = p3ps.tile([128, 512], f32, tag="ps3")
                    nc.tensor.matmul(
                        out=ps[:], lhsT=ones[0:1, 0:128], rhs=bo[0:1, nsl],
                        start=True, stop=False)
                    for mt in range(MT):
                        nc.tensor.matmul(
                            out=ps[:],
                            lhsT=attU[:, mt, qt * 128:(qt + 1) * 128],
                            rhs=wo[:, mt, nsl],
                            start=False, stop=(mt == MT - 1))
                    nc.vector.tensor_scalar(
                        ob[:, nsl], ps[:], mask01[:, qt:qt + 1], None,
                        op0=OP.mult)
                nc.sync.dma_start(out=out_d.ap()[qt * 128:(qt + 1) * 128, :],
                                  in_=ob[:])

    nc.compile()
    return nc


def _get_nc():
    if "nc" not in _CACHE:
        _CACHE["nc"] = _build()
    return _CACHE["nc"]


def _prep_inputs(x, mask, freqs, Wq, bq, Wk, bk, Wv, bv, Wo, bo):
    f = np.asarray(freqs, np.float32)[0]              # [S, HEAD_DIM]
    # reference rotates only the first rot_dim=64 channels of the FLAT
    # inner dim -> rows 0-63 of row-tile 0 on the hg=0 core; everything
    # else is identity (cos=1, sin=0).
    cos2 = np.ones((128, S), np.float32)
    sin2 = np.zeros((128, S), np.float32)
    cos2[0:HEAD_DIM] = np.cos(f.T)
    sin2[0:HEAD_DIM] = np.sin(f.T)
    ident = np.ones((128, S), np.float32)
    identz = np.zeros((128, S), np.float32)

    prt = np.zeros((128, 128), np.float16)            # P_rot^T
    i = np.arange(0, 128, 2)
    prt[i + 1, i] = -1.0                              # P_rot[2i, 2i+1] = -1
    prt[i, i + 1] = 1.0                               # P_rot[2i+1, 2i] = +1

    def lhsT_w(w):                                    # [DIM, DSH] -> lhsT tiles
        return np.ascontiguousarray(
            w.reshape(KT, 128, MT, 128).transpose(1, 0, 2, 3)).astype(np.float16)

    def col(b):                                       # [DSH] -> [128, MT]
        return np.ascontiguousarray(b.reshape(MT, 128).T.astype(np.float32))

    in_maps = []
    for b in range(B):
        xT = np.ascontiguousarray(
            np.asarray(x[b], np.float32).T.reshape(KT, 128, S)
            .transpose(1, 0, 2)).astype(np.float16)
        m = np.asarray(mask[b])
        maskb = np.ascontiguousarray(
            np.where(m, 0.0, MASK_NEG).astype(np.float32).reshape(ST, 128).T)
        mask01 = np.ascontiguousarray(
            m.astype(np.float32).reshape(ST, 128).T)
        for hg in range(HG):
            dsl = slice(hg * DSH, (hg + 1) * DSH)
            in_maps.append({
                "xT": xT,
                "wq": lhsT_w(np.asarray(Wq, np.float32)[:, dsl]),
                "wk": lhsT_w(np.asarray(Wk, np.float32)[:, dsl]),
                "wv": np.ascontiguousarray(
                    np.asarray(Wv, np.float32)[:, dsl]
                    .reshape(KT, 128, DSH).transpose(1, 0, 2)).astype(np.float16),
                "wo": np.ascontiguousarray(
                    np.asarray(Wo, np.float32)[dsl, :]
                    .reshape(MT, 128, DIM).transpose(1, 0, 2)).astype(np.float16),
                "bq": col(np.asarray(bq, np.float32)[dsl]),
                "bk": col(np.asarray(bk, np.float32)[dsl]),
                "bv": np.broadcast_to(
                    np.asarray(bv, np.float32)[dsl], (128, DSH)).copy(),
                "bo": (np.asarray(bo, np.float32) * 0.5)[None, :]
                    .astype(np.float16).copy(),
                "cos2": cos2 if hg == 0 else ident,
                "sin2": sin2 if hg == 0 else identz,
                "prt": prt,
                "maskb": maskb, "mask01": mask01,
            })
    return in_maps


def run(trace=False, **inputs):
    from concourse import bass_utils
    if trace:
        _install_ntff_hook()
    nc = _get_nc()
    in_maps = _prep_inputs(**inputs)
    res = bass_utils.run_bass_kernel_spmd(
        nc, in_maps, core_ids=list(range(NCORES)), trace=trace)
    out = np.empty((B, S, DIM), np.float32)
    for b in range(B):
        out[b] = res.results[2 * b]["out"] + res.results[2 * b + 1]["out"]
    return out, res


def kernel(**inputs):
    out, _ = run(trace=False, **inputs)
    return out


def _install_ntff_hook():
    """Register the axon NTFF profiling hook missing from the antenv stub."""
    import sys, types
    try:
        import antenv.axon_hooks  # noqa: F401
        return
    except ImportError:
        pass
    from trn_agent_boot.trn_boot import _ntff_profile_via_ctypes
    hook = _ntff_profile_via_ctypes('/opt/axon/libaxon_pjrt.so')
    mod = types.ModuleType('antenv.axon_hooks')
    mod.get_axon_ntff_profile_hook = lambda: hook
    mod.set_axon_ntff_profile_hook = lambda h: None
    sys.modules['antenv.axon_hooks'] = mod
